# revision 14
# baseline (speedup 1.0000x reference)
"""Marching-tetrahedra (DiffMC) Trainium2 kernel.

Distribution: voxel dimension (65^3 flattened) split evenly across 8 cores
(sharding per the slab/halo hint, realized as host-side slicing since the
host holds full inputs). Each core runs an identical SPMD Bass program over
its ~34k voxels: per-voxel sign bits + crossing flags, 19 shared edge
interpolations, then per-tet triangle-slot selection via predicated writes
that exploit the complement symmetry of the marching-tets table.

Self-contained: shapes/constants hardcoded for the 64^3 fp32 problem.
"""
import numpy as np

# ---------------- problem constants ----------------
DIM = 64
NV = DIM + 1                 # voxels per axis = 65
VOX = NV * NV * NV           # 274625
NCORES = 8
F = 269                      # voxels per partition per core
NPC = 128 * F                # 34432 voxels per core (8*NPC = 275456 >= VOX)
OFF = [(0, 0, 0), (1, 0, 0), (1, 1, 0), (0, 1, 0),
       (0, 0, 1), (1, 0, 1), (1, 1, 1), (0, 1, 1)]
RING = [1, 2, 3, 7, 4, 5]    # x_t = RING[t], y_t = RING[(t+1)%6]
NS = 32                      # streams: v0..v7, then q[c][xyz] c-major
M_TETS = VOX * 6
N_TRIS = M_TETS * 2

_CACHE = {}


def _streams(grid, deform):
    """Build the 32 per-voxel corner streams, padded to NCORES*NPC voxels."""
    f32 = np.float32
    Gp = np.pad(np.asarray(grid, f32), 1, constant_values=f32(1.0))
    Dp = np.pad(np.asarray(deform, f32), ((1, 1), (1, 1), (1, 1), (0, 0)))
    ax = [np.arange(s, dtype=f32) for s in (DIM + 2, DIM + 2, DIM + 2)]
    coords = np.stack(np.meshgrid(*ax, indexing="ij"), axis=-1)
    Q = ((coords - f32(1.0) + Dp) / f32(DIM - 1)).astype(f32)

    total = NCORES * NPC
    S = np.empty((NS, total), f32)
    # padding voxels: values 1.0 (all corners outside -> empty), positions 0
    for c, (dx, dy, dz) in enumerate(OFF):
        vs = Gp[dx:dx + NV, dy:dy + NV, dz:dz + NV].reshape(-1)
        S[c, :VOX] = vs
        S[c, VOX:] = f32(1.0)
        qs = Q[dx:dx + NV, dy:dy + NV, dz:dz + NV, :].reshape(-1, 3)
        for k in range(3):
            S[8 + 3 * c + k, :VOX] = qs[:, k]
            S[8 + 3 * c + k, VOX:] = f32(0.0)
    return S


def _pack_core(S, r):
    """[NS, total] -> per-core input [128, NS*F] with stream q at cols q*F."""
    seg = S[:, r * NPC:(r + 1) * NPC]          # [NS, NPC]
    seg = seg.reshape(NS, 128, F)              # [NS, 128, F]
    return np.ascontiguousarray(seg.transpose(1, 0, 2).reshape(128, NS * F))


def _split_waits(nc):
    """This environment's walrus accepts at most one on_wait per instruction;
    move extra waits onto preceding same-engine NoOps."""
    import bass_rust
    nsplit = 0
    for f in nc.m.functions:
        for blk in f.blocks:
            newl = []
            for inst in blk.instructions:
                si = inst.sync_info
                w = list(si.on_wait or []) if si is not None else []
                if len(w) > 1:
                    for k, extra in enumerate(w[:-1]):
                        nop = bass_rust.InstNoOp(
                            name=f"{inst.name}_w{k}", ins=[], outs=[])
                        nop.engine = inst.engine
                        nop.sync_info = bass_rust.SyncInfo(
                            on_wait=[extra], on_update=[])
                        newl.append(nop)
                        nsplit += 1
                    inst.sync_info = bass_rust.SyncInfo(
                        on_wait=[w[-1]], on_update=list(si.on_update or []))
                newl.append(inst)
            blk.instructions = newl
    return nsplit


def _build_nc():
    import concourse.bass as bass
    import concourse.mybir as mybir
    from concourse.tile import TileContext

    f32 = mybir.dt.float32
    u8 = mybir.dt.uint8
    A = mybir.AluOpType
    AF = mybir.ActivationFunctionType

    nc = bass.Bass()
    inp = nc.dram_tensor("inp", [128, NS * F], f32, kind="ExternalInput")
    overts = nc.dram_tensor("overts", [6, 128, 20 * F], f32, kind="ExternalOutput")

    def bc3(ap):
        """broadcast a [128, F] AP to [128, 3, F] via a zero-stride middle dim"""
        (ps, pc), (fs, fc) = ap.ap[0], ap.ap[1]
        return bass.AP(ap.tensor, ap.offset, [[ps, pc], [0, 3], [fs, fc]])

    with TileContext(nc) as tc:
        with tc.tile_pool(name="persist", bufs=1) as PP, \
             tc.tile_pool(name="outp", bufs=2) as PO, \
             tc.tile_pool(name="ttmp", bufs=1) as PT:

            # ---- persistent per-voxel tiles ----
            bits = {}          # b0, bb[c], chiA[c], chiR[t], g6, nF
            pos = {}           # edge name -> [128, 3, F]
            selD = PP.tile([128, 3, F], f32, name="selD")

            with tc.tile_pool(name="stage_ab", bufs=1) as PA, \
                 tc.tile_pool(name="edge_tmp", bufs=2) as PE:
                in_mega = PA.tile([128, NS * F], f32, name="in_mega")
                nc.sync.dma_start(in_mega[:], inp[:])

                def V(c):
                    return in_mega[:, c * F:(c + 1) * F]

                def Q3(c):
                    s = (8 + 3 * c) * F
                    ap = in_mega[:, s:s + 3 * F]
                    return bass.AP(ap.tensor, ap.offset,
                                   [ap.ap[0], [F, 3], [1, F]])

                # ---- stage A: sign bits & crossing flags ----
                g6 = PP.tile([128, F], u8, name="g6")
                nc.vector.tensor_scalar(g6[:], V(6), 0.0, None, A.is_gt)
                bits["g6"] = g6
                g0 = PA.tile([128, F], u8, name="g0")
                nc.vector.tensor_scalar(g0[:], V(0), 0.0, None, A.is_gt)
                b0 = PP.tile([128, F], u8, name="b0")
                nc.vector.tensor_tensor(b0[:], g0[:], g6[:], A.logical_xor)
                bits["b0"] = b0
                for c in RING:
                    tl = PP.tile([128, F], u8, name=f"bb{c}")
                    nc.vector.scalar_tensor_tensor(
                        tl[:], V(c), 0.0, g6[:], A.is_gt, A.logical_xor)
                    bits[f"bb{c}"] = tl
                    tl2 = PP.tile([128, F], u8, name=f"chiA{c}")
                    nc.vector.scalar_tensor_tensor(
                        tl2[:], V(c), 0.0, g0[:], A.is_gt, A.logical_xor)
                    bits[f"chiA{c}"] = tl2
                for t in range(6):
                    x, y = RING[t], RING[(t + 1) % 6]
                    tl = PP.tile([128, F], u8, name=f"chiR{t}")
                    nc.vector.tensor_tensor(
                        tl[:], bits[f"bb{x}"][:], bits[f"bb{y}"][:], A.logical_xor)
                    bits[f"chiR{t}"] = tl
                nF = PP.tile([128, F], u8, name="nF")
                nc.vector.tensor_scalar(nF[:], g6[:], 0.5, None, A.is_lt)
                bits["nF"] = nF

                # ---- stage B: 19 edge interpolations ----
                edges = {"D": (0, 6)}
                for c in RING:
                    edges[f"A{c}"] = (0, c)
                    edges[f"B{c}"] = (c, 6)
                for t in range(6):
                    edges[f"R{t}"] = (RING[t], RING[(t + 1) % 6])

                for nm, (al, be) in edges.items():
                    den = PE.tile([128, F], f32, name="den", tag="den")
                    nc.gpsimd.tensor_tensor(den[:], V(be), V(al), A.subtract)
                    ab = PE.tile([128, F], f32, name="ab", tag="ab")
                    nc.scalar.activation(ab[:], den[:], AF.Abs)
                    dg = PE.tile([128, F], f32, name="dg", tag="dg")
                    nc.vector.scalar_tensor_tensor(
                        dg[:], ab[:], 1e-30, ab[:], A.is_le, A.add)
                    # 1/|den'| = exp(-ln(|den'|)) on ACT; sign fixed up below
                    lg = PE.tile([128, F], f32, name="lg", tag="lg")
                    nc.scalar.activation(lg[:], dg[:], AF.Ln)
                    rc = PE.tile([128, F], f32, name="rc", tag="rc")
                    nc.scalar.activation(rc[:], lg[:], AF.Exp, scale=-1.0)
                    # w = -sign(den) = 2*(den<0) - 1 ; te = v_al * w * (1/|den'|)
                    sg = PE.tile([128, F], f32, name="sg", tag="sg")
                    nc.vector.tensor_scalar(sg[:], den[:], 0.0, None, A.is_lt)
                    wv = PE.tile([128, F], f32, name="wv", tag="wv")
                    nc.vector.tensor_scalar(wv[:], sg[:], 2.0, -1.0, A.mult, A.add)
                    uv = PE.tile([128, F], f32, name="uv", tag="uv")
                    nc.vector.tensor_tensor(uv[:], V(al), wv[:], A.mult)
                    te = PE.tile([128, F], f32, name="te", tag="te")
                    nc.vector.tensor_tensor(te[:], uv[:], rc[:], A.mult)
                    p = PP.tile([128, 3, F], f32, name=f"pos_{nm}")
                    d3 = PE.tile([128, 3, F], f32, name="d3", tag="d3")
                    nc.gpsimd.tensor_tensor(d3[:], Q3(be), Q3(al), A.subtract)
                    nc.vector.tensor_tensor(p[:], bc3(te[:]), d3[:], A.mult)
                    nc.vector.tensor_tensor(p[:], p[:], Q3(al), A.add)
                    pos[nm] = p

                nc.gpsimd.memset(selD[:], 0.0)
                nc.vector.copy_predicated(selD[:], bc3(bits["b0"][:]), pos["D"][:])
            # stage_ab pool released: in_mega / g / edge temps freed

            # ---- stage C: per-tet slot selection ----
            cp = nc.vector.copy_predicated
            for t in range(6):
                x, y = RING[t], RING[(t + 1) % 6]
                Ax, Ay = pos[f"A{x}"][:], pos[f"A{y}"][:]
                Bx, By = pos[f"B{x}"][:], pos[f"B{y}"][:]
                R, D = pos[f"R{t}"][:], pos["D"][:]
                b0, b1, b2 = bits["b0"][:], bits[f"bb{x}"][:], bits[f"bb{y}"][:]
                chi1, chiR = bits[f"chiA{x}"][:], bits[f"chiR{t}"][:]

                m = PT.tile([128, F], u8, name="m", tag="m")
                nc.vector.scalar_tensor_tensor(m[:], b2, 0.5, b0, A.is_lt, A.mult)
                b1m = PT.tile([128, F], u8, name="b1m", tag="b1m")
                nc.vector.tensor_tensor(b1m[:], b1, m[:], A.mult)
                b0b1 = PT.tile([128, F], u8, name="b0b1", tag="b0b1")
                nc.vector.tensor_tensor(b0b1[:], b0, b1, A.mult)
                b0b2 = PT.tile([128, F], u8, name="b0b2", tag="b0b2")
                nc.vector.tensor_tensor(b0b2[:], b0, b2, A.mult)
                s2 = PT.tile([128, F], u8, name="s2", tag="s2")
                nc.vector.tensor_tensor(s2[:], b1, b2, A.add)
                s3v = PT.tile([128, F], u8, name="s3v", tag="s3v")
                nc.vector.tensor_tensor(s3v[:], s2[:], b0, A.add)
                P2 = PT.tile([128, F], u8, name="P2", tag="P2")
                nc.vector.tensor_scalar(P2[:], s3v[:], 2.0, None, A.is_equal)
                P2F = PT.tile([128, F], u8, name="P2F", tag="P2F")
                nc.vector.tensor_tensor(P2F[:], P2[:], bits["g6"][:], A.mult)
                P2nF = PT.tile([128, F], u8, name="P2nF", tag="P2nF")
                nc.vector.tensor_tensor(P2nF[:], P2[:], bits["nF"][:], A.mult)
                valid1 = PT.tile([128, F], u8, name="valid1", tag="valid1")
                nc.vector.tensor_tensor(valid1[:], chi1, chiR, A.max)
                nc.vector.tensor_tensor(valid1[:], valid1[:], b0, A.max)

                out_t = PO.tile([128, 20, F], f32, name="out_t", tag="out")

                def slot(s):
                    return out_t[:, 3 * s:3 * s + 3, :]

                # slot0 = chi1 ? Ax : chiR ? Ay : (b0 ? D : 0)
                nc.gpsimd.tensor_copy(slot(0), selD[:])
                cp(slot(0), bc3(chiR), Ay)
                cp(slot(0), bc3(chi1), Ax)
                # s1v (pre-swap slot1): b1 ? (b0&~b2 ? D : Bx) : b2 ? R : b0 ? Ay : 0
                s1v = PT.tile([128, 3, F], f32, name="s1v", tag="s1v")
                nc.gpsimd.memset(s1v[:], 0.0)
                cp(s1v[:], bc3(b0), Ay)
                cp(s1v[:], bc3(b2), R)
                cp(s1v[:], bc3(b1), Bx)
                cp(s1v[:], bc3(b1m[:]), D)
                # s2v (pre-swap slot2): b2 ? By : b1 ? (b0 ? Bx : R) : (b0 ? D : 0)
                s2vt = PT.tile([128, 3, F], f32, name="s2vt", tag="s2vt")
                nc.gpsimd.tensor_copy(s2vt[:], selD[:])
                cp(s2vt[:], bc3(b1), R)
                cp(s2vt[:], bc3(b0b1[:]), Bx)
                cp(s2vt[:], bc3(b2), By)
                # slots 1,2 swap under F=g6
                nc.gpsimd.tensor_copy(slot(1), s1v[:])
                cp(slot(1), bc3(bits["g6"][:]), s2vt[:])
                nc.gpsimd.tensor_copy(slot(2), s2vt[:])
                cp(slot(2), bc3(bits["g6"][:]), s1v[:])
                # slot3 = P2 ? slot0 : 0
                nc.gpsimd.memset(slot(3), 0.0)
                cp(slot(3), bc3(P2[:]), slot(0))
                # c4 = m ? Bx : By ; c5 = b0 ? (b2 ? D : R) : Ay
                c4 = PT.tile([128, 3, F], f32, name="c4", tag="c4")
                nc.gpsimd.tensor_copy(c4[:], By)
                cp(c4[:], bc3(m[:]), Bx)
                c5 = PT.tile([128, 3, F], f32, name="c5", tag="c5")
                nc.gpsimd.tensor_copy(c5[:], Ay)
                cp(c5[:], bc3(b0), R)
                cp(c5[:], bc3(b0b2[:]), D)
                # slots 4,5: swap(c4,c5) under F, gated by P2
                nc.gpsimd.memset(slot(4), 0.0)
                cp(slot(4), bc3(P2nF[:]), c4[:])
                cp(slot(4), bc3(P2F[:]), c5[:])
                nc.gpsimd.memset(slot(5), 0.0)
                cp(slot(5), bc3(P2F[:]), c4[:])
                cp(slot(5), bc3(P2nF[:]), c5[:])

                nc.vector.tensor_copy(out_t[:, 18, :], valid1[:])
                nc.vector.tensor_copy(out_t[:, 19, :], P2[:])
                nc.sync.dma_start(overts[t, :, :], out_t[:])
    _split_waits(nc)
    return nc


def _run_device(S):
    from concourse.bass_utils import run_bass_kernel_spmd
    if "nc" not in _CACHE:
        _CACHE["nc"] = _build_nc()
    nc = _CACHE["nc"]
    in_maps = [{"inp": _pack_core(S, r)} for r in range(NCORES)]
    res = run_bass_kernel_spmd(nc, in_maps, core_ids=list(range(NCORES)))
    return res.results


def kernel(grid, deform):
    S = _streams(grid, deform)
    results = _run_device(S)

    # reassemble: per core overts [6, 128, 18*F] -> [nvox_local, 6, 6, 3]
    verts = np.empty((NCORES * NPC, 6, 6, 3), np.float32)
    valid = np.empty((NCORES * NPC, 6), bool)
    p2 = np.empty((NCORES * NPC, 6), bool)
    for r, out in enumerate(results):
        ov = out["overts"].reshape(6, 128, 20, F)        # [t, p, sc, f]
        vv = ov[:, :, :18, :].reshape(6, 128, 6, 3, F)
        verts[r * NPC:(r + 1) * NPC] = vv.transpose(1, 4, 0, 2, 3).reshape(NPC, 6, 6, 3)
        valid[r * NPC:(r + 1) * NPC] = (ov[:, :, 18, :] != 0).transpose(1, 2, 0).reshape(NPC, 6)
        p2[r * NPC:(r + 1) * NPC] = (ov[:, :, 19, :] != 0).transpose(1, 2, 0).reshape(NPC, 6)

    verts = verts[:VOX].reshape(-1, 3, 3)                # [2M, 3, 3]
    mask = np.stack([valid[:VOX], p2[:VOX]], axis=2).reshape(-1) != 0
    tris = np.arange(N_TRIS * 3, dtype=np.int32).reshape(-1, 3)
    return verts, tris, mask
